# revision 13
# baseline (speedup 1.0000x reference)
"""GuidedAttentionLoss on 8 Trainium2 NeuronCores.

Math: loss = mean_b( sum_{f<F_b, l<L_b} A[b,f,l] * w[b,f,l] / F_b ),
      w = 1 - exp(-c*(l/L - f/F)^2),  c = 1/(2*gamma^(2*step)).

Key identity: exp(-c(x-y)^2) = exp(-cx^2)*exp(-cy^2)*exp(2cxy), and
exp(z) on z in [0, 2c) is approximated by a degree-D polynomial, so the
Gaussian weight is separable:  e[f,l] = sum_k h_k[f] * g_k[l]  with
  h_k[f] = a_k * (2c*y)^k * exp(-c*y^2),  y = f/F   (k = 0..D)
  g_k[l] = x^k * exp(-c*x^2),             x = l/L.
Then sum_{f,l} A*e = sum_k sum_l g_k[l] * C[k,l] with
  C[k,l] = sum_f h_k[f] * A[f,l]   -- a tiny-M matmul Hc^T @ A
(an extra all-ones column of Hc gives sum_f A for the "1" term).

So the whole device kernel is: stream A row-chunks through the
TensorEngine against a small [128 x M] stationary weight, PSUM-
accumulating a [M x L] result per batch; the host does a tiny [M x L]
f64 epilogue. Pure DMA + matmul.

Precision: A is staged to the device as bf16 (element rounding is
unbiased and averages out over the 2048-row contraction; measured
~4e-6 on the final loss). The weights h are split hi/lo into two bf16
columns each (h = hi + lo), recovering ~fp32 weight precision at no
matmul cost (cost scales with N, not with the column count M).

Sharding: pure data parallel over batch. 64 batches -> 8 slots x 8
cores (SPMD: one program, per-core weights/data differ). Batches are
sorted by cost and dealt round-robin so each slot's max (rows, L) is
tight; the program only touches A[:, :ceil(F/128)*128, :Lpad] per slot.
"""

import numpy as np
import ml_dtypes

import concourse.bass as bass  # noqa: F401
import concourse.tile as tile
from concourse import bacc, mybir
from concourse.bass_utils import run_bass_kernel_spmd

B, T_DEC, T_ENC = 64, 2048, 512
G_STEPS, GAMMA = 20000, 0.99995
N_CORES = 8
SLOTS = B // N_CORES
GRP = 8  # 128-row chunks per dma_start (~1 MiB bf16 per transfer)

BF16 = ml_dtypes.bfloat16


def _fit_exp_poly(zmax: float) -> np.ndarray:
    """Monomial coefficients a_k with exp(z) ~= sum a_k z^k on [0, zmax]."""
    from numpy.polynomial import chebyshev as C

    zs = np.linspace(0.0, zmax, 4001)
    ez = np.exp(zs)
    for deg in range(8, 31, 2):
        a = C.cheb2poly(C.chebfit(zs, ez, deg))
        err = np.max(np.abs(np.polynomial.polynomial.polyval(zs, a) - ez))
        if err < 1e-9 * np.exp(zmax):
            return a
    return a


def _plan(input_lengths: np.ndarray, target_lengths: np.ndarray):
    """Assign 64 batches to 8 slots x 8 cores, minimizing per-slot max work.

    Tries two sort keys and keeps whichever yields fewer total bytes.
    (Even free dim is an ISA requirement for the matmul moving operand;
    pad L to a multiple of 4.)
    """
    F = target_lengths.astype(np.int64)
    L = input_lengths.astype(np.int64)
    chunks = (F + 127) // 128

    Lp = -4 * (-L // 4)

    def mk(order):
        sb = np.stack([order[i * N_CORES:(i + 1) * N_CORES]
                       for i in range(SLOTS)])
        return cost(sb), sb

    def cost(sb):
        return int((chunks[sb].max(1) * Lp[sb].max(1)).sum())

    cand1 = mk(np.argsort(-(chunks * L), kind="stable"))
    cand2 = mk(np.lexsort((-L, -chunks)))  # chunks primary, L secondary
    best, assign = min(cand1, cand2, key=lambda t: t[0])
    sb = [assign[i] for i in range(SLOTS)]
    sc = [int(chunks[s].max()) for s in sb]
    sl = [min(T_ENC, -4 * (-int(L[s].max()) // 4)) for s in sb]
    return sb, sc, sl


def _build_program(slot_chunks, slot_L, M):
    f32 = mybir.dt.float32
    bf = mybir.dt.bfloat16
    total_chunks = sum(slot_chunks)
    offs = np.concatenate([[0], np.cumsum(slot_chunks)]).astype(int)

    nc = bacc.Bacc(
        "TRN2", target_bir_lowering=False, debug=False, num_devices=N_CORES
    )
    a_dr = [
        nc.dram_tensor(f"a{i}", [slot_chunks[i] * 128, slot_L[i]], bf,
                       kind="ExternalInput")
        for i in range(SLOTS)
    ]
    h_dr = nc.dram_tensor("h", [128, total_chunks, M], bf,
                          kind="ExternalInput")
    c_dr = [
        nc.dram_tensor(f"c{i}", [M, slot_L[i]], f32, kind="ExternalOutput")
        for i in range(SLOTS)
    ]

    with tile.TileContext(nc) as tc:
        with (
            tc.tile_pool(name="hp", bufs=1) as hpool,
            tc.tile_pool(name="ap", bufs=8) as apool,
            tc.tile_pool(name="op", bufs=2) as opool,
            tc.tile_pool(name="pp", bufs=2, space="PSUM") as pspool,
        ):
            ht = hpool.tile([128, total_chunks, M], bf)
            nc.gpsimd.dma_start(ht[:, :, :], h_dr[:, :, :])
            for i in range(SLOTS):
                nch = slot_chunks[i]
                Lm = slot_L[i]
                ps = pspool.tile([M, Lm], f32, tag="ps")
                bounds = list(range(0, nch, GRP)) + [nch]
                for g0, g1 in zip(bounds, bounds[1:]):
                    gn = g1 - g0
                    at = apool.tile([128, GRP, 512], bf, tag="a")
                    src = a_dr[i][g0 * 128:g1 * 128, :].rearrange(
                        "(g p) l -> p g l", p=128
                    )
                    nc.sync.dma_start(at[:, :gn, :Lm], src)
                    for k in range(gn):
                        ch = g0 + k
                        nc.tensor.matmul(
                            ps[:, :],
                            ht[:, offs[i] + ch, :],
                            at[:, k, :Lm],
                            start=(ch == 0),
                            stop=(ch == nch - 1),
                        )
                ot = opool.tile([M, Lm], f32, tag="o")
                nc.scalar.copy(ot[:, :], ps[:, :])
                nc.gpsimd.dma_start(c_dr[i][:, :], ot[:, :])
    nc.compile()
    return nc


def _kernel_impl(alignments, input_lengths, target_lengths, global_step,
                 trace=False):
    step = int(global_step)
    if G_STEPS < step:
        return np.zeros((), dtype=np.float32), None

    g = GAMMA ** step
    c = 1.0 / (2.0 * g * g)
    a_poly = _fit_exp_poly(2.0 * c)
    D = len(a_poly) - 1
    # weight columns: [hi_0..hi_D, ones, lo_0..lo_D]
    M = 2 * (D + 1) + 1

    F = target_lengths.astype(np.int64)
    L = input_lengths.astype(np.int64)
    slot_batches, slot_chunks, slot_L = _plan(input_lengths, target_lengths)
    offs = np.concatenate([[0], np.cumsum(slot_chunks)]).astype(int)
    total_chunks = int(offs[-1])

    nc = _build_program(slot_chunks, slot_L, M)

    al = np.asarray(alignments, dtype=np.float32)
    in_maps = []
    for j in range(N_CORES):
        im = {}
        h_all = np.zeros((total_chunks, 128, M), dtype=BF16)
        for i in range(SLOTS):
            b = int(slot_batches[i][j])
            R = slot_chunks[i] * 128
            Lm = slot_L[i]
            im[f"a{i}"] = al[b, :R, :Lm].astype(BF16)
            Fb = int(F[b])
            y = np.arange(R, dtype=np.float64) / Fb
            h = np.zeros((R, D + 2), dtype=np.float64)
            for k in range(D + 1):
                h[:, k] = a_poly[k] * (2.0 * c * y) ** k * np.exp(-c * y * y)
            h[:, D + 1] = 1.0
            h[Fb:, :] = 0.0
            hi = h.astype(BF16)
            lo = (h - hi.astype(np.float64)).astype(BF16)
            blk = h_all[offs[i]:offs[i + 1]].reshape(R, M)
            blk[:, :D + 2] = hi
            blk[:, D + 2:] = lo[:, :D + 1]
        im["h"] = np.ascontiguousarray(h_all.transpose(1, 0, 2))
        in_maps.append(im)

    res = run_bass_kernel_spmd(nc, in_maps, list(range(N_CORES)), trace=trace)

    # Host epilogue: tiny [M, L] combinations per batch, f64.
    per_sample = np.zeros(B, dtype=np.float64)
    for j in range(N_CORES):
        for i in range(SLOTS):
            b = int(slot_batches[i][j])
            Lb = int(L[b])
            Cm = res.results[j][f"c{i}"].astype(np.float64)
            Ck = Cm[:D + 1, :Lb] + Cm[D + 2:, :Lb]  # hi + lo
            x = np.arange(Lb, dtype=np.float64) / Lb
            ex = np.exp(-c * x * x)
            gsum = np.zeros(Lb)
            xk = np.ones(Lb)
            for k in range(D + 1):
                gsum += Ck[k] * xk
                xk *= x
            per_sample[b] = Cm[D + 1, :Lb].sum() - (gsum * ex).sum()
    loss = np.float64(np.mean(per_sample / F.astype(np.float64)))
    return np.asarray(loss, dtype=np.float32), res


def kernel(alignments, input_lengths, target_lengths, global_step):
    loss, _ = _kernel_impl(alignments, input_lengths, target_lengths,
                           global_step)
    return loss


# revision 14
# speedup vs baseline: 1.0458x; 1.0458x over previous
"""GuidedAttentionLoss on 8 Trainium2 NeuronCores.

Math: loss = mean_b( sum_{f<F_b, l<L_b} A[b,f,l] * w[b,f,l] / F_b ),
      w = 1 - exp(-c*(l/L - f/F)^2),  c = 1/(2*gamma^(2*step)).

Key identity: exp(-c(x-y)^2) = exp(-cx^2)*exp(-cy^2)*exp(2cxy), and
exp(z) on z in [0, 2c) is approximated by a degree-D polynomial, so the
Gaussian weight is separable:  e[f,l] = sum_k h_k[f] * g_k[l]  with
  h_k[f] = a_k * (2c*y)^k * exp(-c*y^2),  y = f/F   (k = 0..D)
  g_k[l] = x^k * exp(-c*x^2),             x = l/L.
Then sum_{f,l} A*e = sum_k sum_l g_k[l] * C[k,l] with
  C[k,l] = sum_f h_k[f] * A[f,l]   -- a tiny-M matmul Hc^T @ A
(an extra all-ones column of Hc gives sum_f A for the "1" term).

So the whole device kernel is: stream A row-chunks through the
TensorEngine against a small [128 x M] stationary weight, PSUM-
accumulating a [M x L] result per batch; the host does a tiny [M x L]
f64 epilogue. Pure DMA + matmul.

Precision: A is staged to the device as bf16 (element rounding is
unbiased and averages out over the 2048-row contraction; measured
~4e-6 on the final loss). The weights h are split hi/lo into two bf16
columns each (h = hi + lo), recovering ~fp32 weight precision at no
matmul cost (cost scales with N, not with the column count M).

Sharding: pure data parallel over batch. 64 batches -> 8 slots x 8
cores (SPMD: one program, per-core weights/data differ). Batches are
sorted by cost and dealt round-robin so each slot's max (rows, L) is
tight; the program only touches A[:, :ceil(F/128)*128, :Lpad] per slot.
"""

import numpy as np
import ml_dtypes

import concourse.bass as bass  # noqa: F401
import concourse.tile as tile
from concourse import bacc, mybir
from concourse.bass_utils import run_bass_kernel_spmd

B, T_DEC, T_ENC = 64, 2048, 512
G_STEPS, GAMMA = 20000, 0.99995
N_CORES = 8
SLOTS = B // N_CORES
GRP = 8  # 128-row chunks per dma_start (~1 MiB bf16 per transfer)

BF16 = ml_dtypes.bfloat16


def _fit_exp_poly(zmax: float) -> np.ndarray:
    """Monomial coefficients a_k with exp(z) ~= sum a_k z^k on [0, zmax]."""
    from numpy.polynomial import chebyshev as C

    zs = np.linspace(0.0, zmax, 4001)
    ez = np.exp(zs)
    for deg in range(8, 31, 2):
        a = C.cheb2poly(C.chebfit(zs, ez, deg))
        err = np.max(np.abs(np.polynomial.polynomial.polyval(zs, a) - ez))
        if err < 1e-9 * np.exp(zmax):
            return a
    return a


def _plan(input_lengths: np.ndarray, target_lengths: np.ndarray):
    """Assign 64 batches to 8 slots x 8 cores, minimizing per-slot max work.

    Tries two sort keys and keeps whichever yields fewer total bytes.
    (Even free dim is an ISA requirement for the matmul moving operand;
    pad L to a multiple of 4.)
    """
    F = target_lengths.astype(np.int64)
    L = input_lengths.astype(np.int64)
    chunks = (F + 127) // 128

    Lp = -4 * (-L // 4)

    def mk(order):
        sb = np.stack([order[i * N_CORES:(i + 1) * N_CORES]
                       for i in range(SLOTS)])
        return cost(sb), sb

    def cost(sb):
        return int((chunks[sb].max(1) * Lp[sb].max(1)).sum())

    cand1 = mk(np.argsort(-(chunks * L), kind="stable"))
    cand2 = mk(np.lexsort((-L, -chunks)))  # chunks primary, L secondary
    best, assign = min(cand1, cand2, key=lambda t: t[0])
    sb = [assign[i] for i in range(SLOTS)]
    sc = [int(chunks[s].max()) for s in sb]
    sl = [min(T_ENC, -4 * (-int(L[s].max()) // 4)) for s in sb]
    return sb, sc, sl


def _build_program(slot_chunks, slot_L, M):
    f32 = mybir.dt.float32
    bf = mybir.dt.bfloat16
    total_chunks = sum(slot_chunks)
    offs = np.concatenate([[0], np.cumsum(slot_chunks)]).astype(int)

    nc = bacc.Bacc(
        "TRN2", target_bir_lowering=False, debug=False, num_devices=N_CORES
    )
    a_dr = [
        nc.dram_tensor(f"a{i}", [slot_chunks[i] * 128, slot_L[i]], bf,
                       kind="ExternalInput")
        for i in range(SLOTS)
    ]
    h_dr = nc.dram_tensor("h", [128, total_chunks, M], bf,
                          kind="ExternalInput")
    c_dr = [
        nc.dram_tensor(f"c{i}", [M, slot_L[i]], f32, kind="ExternalOutput")
        for i in range(SLOTS)
    ]

    with tile.TileContext(nc) as tc:
        with (
            tc.tile_pool(name="hp", bufs=1) as hpool,
            tc.tile_pool(name="ap", bufs=6) as apool,
            tc.tile_pool(name="op", bufs=2) as opool,
            tc.tile_pool(name="pp", bufs=2, space="PSUM") as pspool,
        ):
            ht = hpool.tile([128, total_chunks, M], bf)
            nc.gpsimd.dma_start(ht[:, :, :], h_dr[:, :, :])
            for i in range(SLOTS):
                nch = slot_chunks[i]
                Lm = slot_L[i]
                ps = pspool.tile([M, Lm], f32, tag="ps")
                bounds = list(range(0, nch, GRP)) + [nch]
                for g0, g1 in zip(bounds, bounds[1:]):
                    gn = g1 - g0
                    at = apool.tile([128, GRP, 512], bf, tag="a")
                    src = a_dr[i][g0 * 128:g1 * 128, :].rearrange(
                        "(g p) l -> p g l", p=128
                    )
                    nc.sync.dma_start(at[:, :gn, :Lm], src)
                    for k in range(gn):
                        ch = g0 + k
                        nc.tensor.matmul(
                            ps[:, :],
                            ht[:, offs[i] + ch, :],
                            at[:, k, :Lm],
                            start=(ch == 0),
                            stop=(ch == nch - 1),
                        )
                ot = opool.tile([M, Lm], f32, tag="o")
                nc.scalar.copy(ot[:, :], ps[:, :])
                nc.gpsimd.dma_start(c_dr[i][:, :], ot[:, :])
    nc.compile()
    return nc


def _kernel_impl(alignments, input_lengths, target_lengths, global_step,
                 trace=False):
    step = int(global_step)
    if G_STEPS < step:
        return np.zeros((), dtype=np.float32), None

    g = GAMMA ** step
    c = 1.0 / (2.0 * g * g)
    a_poly = _fit_exp_poly(2.0 * c)
    D = len(a_poly) - 1
    # weight columns: [hi_0..hi_D, ones, lo_0..lo_D]
    M = 2 * (D + 1) + 1

    F = target_lengths.astype(np.int64)
    L = input_lengths.astype(np.int64)
    slot_batches, slot_chunks, slot_L = _plan(input_lengths, target_lengths)
    offs = np.concatenate([[0], np.cumsum(slot_chunks)]).astype(int)
    total_chunks = int(offs[-1])

    nc = _build_program(slot_chunks, slot_L, M)

    al = np.asarray(alignments, dtype=np.float32)
    in_maps = []
    for j in range(N_CORES):
        im = {}
        h_all = np.zeros((total_chunks, 128, M), dtype=BF16)
        for i in range(SLOTS):
            b = int(slot_batches[i][j])
            R = slot_chunks[i] * 128
            Lm = slot_L[i]
            im[f"a{i}"] = al[b, :R, :Lm].astype(BF16)
            Fb = int(F[b])
            y = np.arange(R, dtype=np.float64) / Fb
            h = np.zeros((R, D + 2), dtype=np.float64)
            for k in range(D + 1):
                h[:, k] = a_poly[k] * (2.0 * c * y) ** k * np.exp(-c * y * y)
            h[:, D + 1] = 1.0
            h[Fb:, :] = 0.0
            hi = h.astype(BF16)
            lo = (h - hi.astype(np.float64)).astype(BF16)
            blk = h_all[offs[i]:offs[i + 1]].reshape(R, M)
            blk[:, :D + 2] = hi
            blk[:, D + 2:] = lo[:, :D + 1]
        im["h"] = np.ascontiguousarray(h_all.transpose(1, 0, 2))
        in_maps.append(im)

    res = run_bass_kernel_spmd(nc, in_maps, list(range(N_CORES)), trace=trace)

    # Host epilogue: tiny [M, L] combinations per batch, f64.
    per_sample = np.zeros(B, dtype=np.float64)
    for j in range(N_CORES):
        for i in range(SLOTS):
            b = int(slot_batches[i][j])
            Lb = int(L[b])
            Cm = res.results[j][f"c{i}"].astype(np.float64)
            Ck = Cm[:D + 1, :Lb] + Cm[D + 2:, :Lb]  # hi + lo
            x = np.arange(Lb, dtype=np.float64) / Lb
            ex = np.exp(-c * x * x)
            gsum = np.zeros(Lb)
            xk = np.ones(Lb)
            for k in range(D + 1):
                gsum += Ck[k] * xk
                xk *= x
            per_sample[b] = Cm[D + 1, :Lb].sum() - (gsum * ex).sum()
    loss = np.float64(np.mean(per_sample / F.astype(np.float64)))
    return np.asarray(loss, dtype=np.float32), res


def kernel(alignments, input_lengths, target_lengths, global_step):
    loss, _ = _kernel_impl(alignments, input_lengths, target_lengths,
                           global_step)
    return loss
